# revision 2
# baseline (speedup 1.0000x reference)
"""Trainium2 Bass kernel for nn_MoEConnectionProcessor (v2).

Math (per row b, D=64, K=26):
  masks from tier (0=local,1=func,2=dist)
  agg_l = masked_mean(ns, tier==0); h_local = tanh([cs,agg_l]@W_local)
  msg = relu(ns@W2 + (cs@W1 + b_msg)) per (b,k); agg_f = masked_mean(msg, tier==1)
  h = tanh([cs,agg_f]@W_upd); 3x Euler: h += .1*tanh(h@W_fcnf)
  agg_d = masked_mean(ns, tier==2); h_dist=cs; 3x: h += .1*tanh([h,agg_d]@W_dcnf)
  gates = softmax(relu([cs, mean_k ns]@W_g1)@W_g2); out = sum_k g_k * h_k

Strategy (data parallel, Bc=4096/core):
  - ns shipped bf16 twice: token-major (for mask-weighted aggregation matmuls,
    contraction over the 104 tokens of a 4-row tile) and pair-major, which a
    single InstDmaTransposeAnt per 4-chunk group turns directly into the
    D-major lhsT the msg matmul needs (no PE transposes, no PSUM evacuation).
  - msg matmul runs 2 tiles at a time via a block-diagonal [[W2,0],[0,W2]]
    rhs on the full 128 contraction; the (cs@W1+b) broadcast term is added by
    matmuls whose lhsT is a stride-0-broadcast view of host-transposed csT
    (with a ones row appended so b_msg rides the contraction).
  - per-super expert/gating chain runs b-split [128,256] (two 256-col halves
    on partition halves) with block-diagonal weights: every matmul streams
    512*?  no - 256 free at 1 cyc/row f32r, every activation covers 128
    partitions.
  - epilogue transposes h back to b-major via accumulating identity matmuls
    that also fold the 0.1*sum Euler terms; gates softmax-combine via
    per-partition scaled activations; one out DMA per 512 rows.
"""

import os
import sys

sys.path.insert(0, "/opt/trn_rl_repo")

import numpy as np
import ml_dtypes

import concourse.bass as bass
import concourse.mybir as mybir
import concourse.tile as tile
from concourse.bass_utils import run_bass_kernel_spmd

F32 = mybir.dt.float32
F32R = mybir.dt.float32r
BF16 = mybir.dt.bfloat16
F8E4 = mybir.dt.float8e4

D = 64
K = 26
GH = 32
NCORES = 8
CB = 64            # b-rows per chunk (16 tiles of 4 rows / 104 tokens)
SB = 512           # b-rows per super (8 chunks)
GC = 4             # chunks per DMA group
AFT = mybir.ActivationFunctionType
ALU = mybir.AluOpType
AXL = mybir.AxisListType

# token-major ns ships fp8 (aggregation lhsT only); masks ride a per-super
# bf16 tensor: per chunk 192 mask cols (3 masks x 4 rows x 16 tiles) + 64 fse
NSF = 1024         # 16 tiles * 64
MKF = 192
FSF = 64
MCH = MKF + FSF    # 256 mask cols per chunk

# wpack (f32) column layout: 9 block-diagonal [128,128] weights,
# 2 [128,64] gating blocks, wg2blk [64,36], then bias columns.
WB = {
    "wl1": 0, "wlb": 128, "wu1": 256, "wub": 384, "wf": 512, "wfs": 640,
    "wda": 768, "wdb": 896, "wds": 1024,
}
WG1A = 1152
WG1B = 1216
WG2 = 1280         # [0:64, 1280:1316]
BG1 = 1316         # [0:64]
BG2 = 1317         # [64:100]  ([bg2;0...;bg2])
BL2 = 1318         # [128,1] [bl;bl]
BU2 = 1319
BFC2 = 1320
BDC2 = 1321
WPC = 1322

# bpack (bf16) column layout
BW2 = 0            # w2blk [128, 0:128]
BW1B = 128         # w1b65 [0:65, 128:192]
BI64 = 192         # identity stacked twice [128, 192:256]
BIP1 = 256         # 0.1*identity stacked twice [128, 256:320]
BEY3 = 320         # eye3 at rows 64:67, cols 320:323
SELB = 324         # 32 selector tiles [128,104]: idx=(ph*8+u)*2+par
BPC = 324 + 32 * 104


def r32(ap):
    return ap.bitcast(F32R)


def _split_waits(nc):
    """Walrus encodes at most one sync-wait command on most TPB instructions.
    Hoist extra on_wait entries into standalone single-wait EventSemaphore
    instructions placed immediately before, on the same engine queue."""
    for f in nc.m.functions:
        for blk in f.blocks:
            insts = list(blk.instructions)
            out = []
            changed = False
            for inst in insts:
                si = inst.sync_info
                ow = list(si.on_wait) if (si is not None and si.on_wait) else []
                if len(ow) > 1:
                    changed = True
                    for w in ow[:-1]:
                        out.append(
                            mybir.InstEventSemaphore(
                                name=nc.get_next_instruction_name(),
                                engine=inst.engine,
                                ins=[],
                                outs=[],
                                sync_info=mybir.SyncInfo(on_wait=[w], on_update=[]),
                            )
                        )
                    inst.sync_info = mybir.SyncInfo(
                        on_wait=[ow[-1]], on_update=list(si.on_update or [])
                    )
                out.append(inst)
            if changed:
                blk.instructions = out


def build_program(Bc, zb=True, use_dmat=True):
    assert Bc % SB == 0
    nsup = Bc // SB
    nch = Bc // CB
    ngr = nch // GC

    nc = bass.Bass(trn_type="TRN2", target_bir_lowering=False, debug=False)

    m8_d = nc.dram_tensor("m8", [ngr, 104, GC * NSF], F8E4, kind="ExternalInput").ap()
    mkf_d = nc.dram_tensor("mkf", [nsup, 104, 8 * MCH], BF16, kind="ExternalInput").ap()
    nst_d = nc.dram_tensor("nst", [ngr, GC * 832, 128], BF16, kind="ExternalInput").ap()
    cstt_d = nc.dram_tensor("cstt", [65, Bc], BF16, kind="ExternalInput").ap()
    cst2_d = nc.dram_tensor("cst2", [128, Bc // 2], BF16, kind="ExternalInput").ap()
    wpack_d = nc.dram_tensor("wpack", [128, WPC], BF16, kind="ExternalInput").ap()
    fpack_d = nc.dram_tensor("fpack", [128, 8], F32, kind="ExternalInput").ap()
    bpack_d = nc.dram_tensor("bpack", [128, BPC], BF16, kind="ExternalInput").ap()
    out_d = nc.dram_tensor("out", [Bc, D], BF16, kind="ExternalOutput").ap()

    out_v = out_d.rearrange("(s w p) d -> s p w d", p=128, w=4)

    with tile.TileContext(nc) as tc:
        with (
            tc.tile_pool(name="sing", bufs=1) as sing,
            tc.tile_pool(name="pns", bufs=4) as pns,
            tc.tile_pool(name="pmb", bufs=4) as pmb,
            tc.tile_pool(name="pmsgr", bufs=2) as pmsgr,
            tc.tile_pool(name="psup", bufs=2) as psup,
            tc.tile_pool(name="pch", bufs=2) as pch,
            tc.tile_pool(name="pot", bufs=4) as pot,
            tc.tile_pool(name="pstg", bufs=2) as pstg,
            tc.tile_pool(name="ppm", bufs=2, space="PSUM") as ppm,
            tc.tile_pool(name="ppag", bufs=2, space="PSUM") as ppag,
            tc.tile_pool(name="ppsc", bufs=2, space="PSUM") as ppsc,
            tc.tile_pool(name="ppe", bufs=2, space="PSUM") as ppe,
            tc.tile_pool(name="ppt", bufs=1, space="PSUM") as ppt,
        ):
            wpack = sing.tile([128, WPC], BF16, tag="wpack")
            fpack = sing.tile([128, 8], F32, tag="fpack")
            nc.sync.dma_start(out=fpack, in_=fpack_d)
            nc.sync.dma_start(out=wpack, in_=wpack_d)
            bpack = sing.tile([128, BPC], BF16, tag="bpack")
            nc.sync.dma_start(out=bpack, in_=bpack_d)

            w2blk = bpack[:, BW2:BW2 + 128]
            w1b65 = bpack[0:65, BW1B:BW1B + 64]
            i64a = bpack[0:64, BI64:BI64 + 64]
            i64b = bpack[64:128, BI64:BI64 + 64]
            ip1a = bpack[0:64, BIP1:BIP1 + 64]
            ip1b = bpack[64:128, BIP1:BIP1 + 64]
            ey3a = bpack[64:67, BEY3:BEY3 + 3]

            # startup observer matmuls so later matmuls carry <=1 wait
            pwu = ppsc.tile([128, 512], F32, tag="psc")
            nc.tensor.matmul(pwu[0:32, 0:32], wpack[0:32, 0:32], wpack[0:32, 0:32], start=True, stop=True)
            nc.tensor.matmul(pwu[0:32, 0:32], bpack[0:32, 0:32], bpack[0:32, 0:32], start=True, stop=True)

            def emit_chunk(s, gg, cc, mbl, nsT, t8, mkf, agg3, aggf2):
                h = gg * GC + cc          # chunk within super
                c0 = cc * NSF             # m8 col base for chunk
                m0 = (gg * GC + cc) * MCH # mkf col base for chunk
                n0 = cc * 832             # nsT col base
                hw_ = h % 4
                P0, P1 = (0, 64) if h < 4 else (64, 128)

                # ---- aggregation matmuls (only need mbl/mkf) ----
                pagg = ppag.tile([128, 256], F32, tag="pagg", bufs=1)
                for q in range(16):
                    nc.tensor.matmul(
                        pagg[P0:P1, 12 * q:12 * q + 12],
                        mbl[:, c0 + 64 * q: c0 + 64 * (q + 1)],
                        mkf[:, m0 + 12 * q: m0 + 12 * (q + 1)],
                        start=True, stop=True,
                    )

                # ---- msg matmuls: ns@W2blk + t-broadcast ----
                pmA = ppm.tile([104, 512], F32, tag="pm")
                pmB = ppm.tile([104, 512], F32, tag="pm")
                ph_ = h % 2
                t8s = t8[:, 64 * (h // 2):64 * (h // 2) + 64]
                for u in range(8):
                    pm = pmA if u < 4 else pmB
                    o0 = 128 * (u % 4)
                    nc.tensor.matmul(
                        pm[:, o0:o0 + 128],
                        nsT[:, n0 + 104 * u: n0 + 104 * (u + 1)],
                        w2blk, start=True, stop=False,
                    )
                    for par in range(2):
                        si = SELB + 104 * (((ph_ * 8 + u) * 2) + par)
                        nc.tensor.matmul(
                            pm[:, o0 + 64 * par: o0 + 64 * par + 64],
                            bpack[:, si:si + 104], t8s,
                            start=False, stop=(par == 1),
                        )
                msgr = pmsgr.tile([104, 1024], BF16, tag="msgr")
                nc.scalar.activation(msgr[:, 0:512], pmA, AFT.Relu)
                nc.vector.tensor_scalar(msgr[:, 512:1024], pmB, 0.0, None, ALU.max)

                for q in range(16):
                    nc.tensor.matmul(
                        pagg[P0:P1, 192 + 4 * q:196 + 4 * q],
                        msgr[:, 64 * q:64 * (q + 1)],
                        mkf[:, m0 + MKF + 4 * q: m0 + MKF + 4 * (q + 1)],
                        start=True, stop=True,
                    )
                nc.vector.tensor_copy(
                    agg3[P0:P1].rearrange("p (m bc) -> p m bc", m=3)
                    [:, :, 64 * hw_:64 * hw_ + 64]
                    .rearrange("p m (q g) -> p q m g", g=4),
                    pagg[P0:P1, 0:192].rearrange("p (q m g) -> p q m g", m=3, g=4),
                )
                nc.vector.tensor_copy(
                    aggf2[P0:P1, 64 * hw_:64 * hw_ + 64],
                    pagg[P0:P1, 192:256],
                )

            W = lambda k: wpack[:, WB[k]:WB[k] + 128]

            def emit_chain(s, cst2, agg3, aggf2):
                """Generator: chain+epilogue for super s; each step keeps the
                number of act-blocked PE matmuls under the 4-deep wait queue."""
                agl = agg3[:, 0:256]
                agd = agg3[:, 256:512]
                agm = agg3[:, 512:768]

                # step 1: everything that only needs the aggregates
                pzg = ppsc.tile([128, 512], F32, tag="psc")
                nc.tensor.matmul(pzg[0:64, 0:256], wpack[:, WG1A:WG1A + 64], cst2, start=True, stop=False)
                nc.tensor.matmul(pzg[0:64, 0:256], wpack[:, WG1B:WG1B + 64], agm, start=False, stop=True)
                g1r = pch.tile([64, 256], BF16, tag="g1r")
                nc.vector.tensor_scalar(
                    g1r, pzg[0:64, 0:256], fpack[0:64, 0:1], 0.0,
                    ALU.add, ALU.max,
                )
                ph = ppsc.tile([128, 512], F32, tag="psc")
                nc.tensor.matmul(ph[:, 0:256], W("wl1"), cst2, start=True, stop=False)
                nc.tensor.matmul(ph[:, 0:256], W("wlb"), agl, start=False, stop=True)
                nc.tensor.matmul(ph[:, 256:512], W("wu1"), cst2, start=True, stop=False)
                nc.tensor.matmul(ph[:, 256:512], W("wub"), aggf2, start=False, stop=True)
                hlu = pch.tile([128, 512], BF16, tag="hlu")
                if zb:
                    nc.scalar.activation(hlu, ph, AFT.Tanh)
                else:
                    nc.scalar.activation(hlu[:, 0:256], ph[:, 0:256], AFT.Tanh, bias=fpack[:, 2:3])
                    nc.scalar.activation(hlu[:, 256:512], ph[:, 256:512], AFT.Tanh, bias=fpack[:, 3:4])
                hl = hlu[:, 0:256]
                hu0 = hlu[:, 256:512]
                yield

                # step 2: logits (g1r ready by now) + tfd0 d-side (ready)
                pex = pzg[:, 384:400]
                exw = pot.tile([128, 12], F32, tag="exw")
                if zb:
                    for w in range(4):
                        lo = 32 * (w >= 2)
                        nc.tensor.matmul(
                            pex[:, 3 * w:3 * w + 3],
                            g1r[lo:lo + 32, 128 * (w % 2):128 * (w % 2) + 128],
                            wpack[lo:lo + 32, WG2 + 3 * (w >= 2):WG2 + 3 * (w >= 2) + 3],
                            start=True, stop=True,
                        )
                    nc.scalar.activation(exw, pex[:, 0:12], AFT.Exp)
                else:
                    nc.tensor.matmul(pzg[64:67, 0:256], wpack[0:64, WG2:WG2 + 3], g1r, start=True, stop=True)
                    nc.tensor.matmul(pzg[64:67, 256:512], wpack[0:64, WG2 + 3:WG2 + 6], g1r, start=True, stop=True)
                    expt = pch.tile([128, 512], BF16, tag="expt")
                    nc.scalar.activation(
                        expt[64:67], pzg[64:67, 0:512], AFT.Exp,
                        bias=fpack[64:67, 1:2],
                    )
                    for w in range(4):
                        eb = 256 * (w >= 2) + 128 * (w % 2)
                        nc.tensor.matmul(pex[:, 3 * w:3 * w + 3], expt[64:67, eb:eb + 128], ey3a, start=True, stop=True)
                    nc.vector.tensor_copy(exw, pex[:, 0:12])
                pz0 = ppsc.tile([128, 512], F32, tag="psc")
                nc.tensor.matmul(pz0[:, 256:512], W("wda"), cst2, start=True, stop=False)
                nc.tensor.matmul(pz0[:, 256:512], W("wdb"), agd, start=False, stop=True)
                yield

                # step 3: tfd0 f-side (waits hlu act) + act + softmax scalars
                nc.tensor.matmul(pz0[:, 0:256], W("wf"), hu0, start=True, stop=True)
                tfd0 = pch.tile([128, 512], BF16, tag="tfd0")
                if zb:
                    nc.scalar.activation(tfd0, pz0, AFT.Tanh)
                else:
                    nc.scalar.activation(tfd0[:, 0:256], pz0[:, 0:256], AFT.Tanh, bias=fpack[:, 4:5])
                    nc.scalar.activation(tfd0[:, 256:512], pz0[:, 256:512], AFT.Tanh, bias=fpack[:, 5:6])
                se = pot.tile([128, 4], F32, tag="se")
                nc.vector.tensor_reduce(
                    se.rearrange("p (w o) -> p w o", o=1),
                    exw.rearrange("p (w k) -> p w k", k=3),
                    AXL.X, ALU.add,
                )
                rc = pot.tile([128, 4], F32, tag="rc")
                nc.vector.reciprocal(rc, se)
                gk = pot.tile([128, 12], F32, tag="gk")
                for w in range(4):
                    nc.vector.tensor_scalar(
                        gk[:, 3 * w:3 * w + 3], exw[:, 3 * w:3 * w + 3],
                        rc[:, w:w + 1], None, ALU.mult,
                    )
                yield

                # steps 4-5: tfd1 (d-side ready mms first, tfd0-waiters later)
                pz1 = ppsc.tile([128, 512], F32, tag="psc")
                nc.tensor.matmul(pz1[:, 256:512], W("wda"), cst2, start=True, stop=False)
                nc.tensor.matmul(pz1[:, 256:512], W("wdb"), agd, start=False, stop=False)
                yield
                nc.tensor.matmul(pz1[:, 256:512], W("wds"), tfd0[:, 256:512], start=False, stop=True)
                nc.tensor.matmul(pz1[:, 0:256], W("wf"), hu0, start=True, stop=False)
                nc.tensor.matmul(pz1[:, 0:256], W("wfs"), tfd0[:, 0:256], start=False, stop=True)
                tfd1 = pch.tile([128, 512], BF16, tag="tfd1")
                if zb:
                    nc.scalar.activation(tfd1, pz1, AFT.Tanh)
                else:
                    nc.scalar.activation(tfd1[:, 0:256], pz1[:, 0:256], AFT.Tanh, bias=fpack[:, 4:5])
                    nc.scalar.activation(tfd1[:, 256:512], pz1[:, 256:512], AFT.Tanh, bias=fpack[:, 5:6])
                yield

                # steps 6-7: tfd2
                pz2 = ppsc.tile([128, 512], F32, tag="psc")
                nc.tensor.matmul(pz2[:, 256:512], W("wda"), cst2, start=True, stop=False)
                nc.tensor.matmul(pz2[:, 256:512], W("wdb"), agd, start=False, stop=False)
                nc.tensor.matmul(pz2[:, 256:512], W("wds"), tfd0[:, 256:512], start=False, stop=False)
                yield
                nc.tensor.matmul(pz2[:, 256:512], W("wds"), tfd1[:, 256:512], start=False, stop=True)
                nc.tensor.matmul(pz2[:, 0:256], W("wf"), hu0, start=True, stop=False)
                nc.tensor.matmul(pz2[:, 0:256], W("wfs"), tfd0[:, 0:256], start=False, stop=False)
                nc.tensor.matmul(pz2[:, 0:256], W("wfs"), tfd1[:, 0:256], start=False, stop=True)
                tfd2 = pch.tile([128, 512], BF16, tag="tfd2")
                if zb:
                    nc.scalar.activation(tfd2, pz2, AFT.Tanh)
                else:
                    nc.scalar.activation(tfd2[:, 0:256], pz2[:, 0:256], AFT.Tanh, bias=fpack[:, 4:5])
                    nc.scalar.activation(tfd2[:, 256:512], pz2[:, 256:512], AFT.Tanh, bias=fpack[:, 5:6])
                tfds = [tfd0, tfd1, tfd2]
                yield

                stg = pstg.tile([128, 256], BF16, tag="stg")
                for w in range(4):
                    hi = w >= 2
                    R0, R1 = (64, 128) if hi else (0, 64)
                    b0 = 128 * (w % 2)
                    i64 = i64b if hi else i64a
                    ip1 = ip1b if hi else ip1a
                    sl = slice(b0, b0 + 128)
                    pe = ppe.tile([128, 256], F32, tag="pe")
                    nc.tensor.matmul(pe[:, 0:64], hl[R0:R1, sl], i64, start=True, stop=True)
                    nc.tensor.matmul(pe[:, 64:128], hu0[R0:R1, sl], i64, start=True, stop=False)
                    nc.tensor.matmul(pe[:, 64:128], tfds[0][R0:R1, sl], ip1, start=False, stop=False)
                    nc.tensor.matmul(pe[:, 64:128], tfds[1][R0:R1, sl], ip1, start=False, stop=False)
                    nc.tensor.matmul(pe[:, 64:128], tfds[2][R0:R1, sl], ip1, start=False, stop=True)
                    nc.tensor.matmul(pe[:, 128:192], cst2[R0:R1, sl], i64, start=True, stop=False)
                    nc.tensor.matmul(pe[:, 128:192], tfds[0][R0:R1, 256 + b0:256 + b0 + 128], ip1, start=False, stop=False)
                    nc.tensor.matmul(pe[:, 128:192], tfds[1][R0:R1, 256 + b0:256 + b0 + 128], ip1, start=False, stop=False)
                    nc.tensor.matmul(pe[:, 128:192], tfds[2][R0:R1, 256 + b0:256 + b0 + 128], ip1, start=False, stop=True)
                    t1 = pot.tile([128, 64], F32, tag="t1")
                    nc.scalar.activation(t1, pe[:, 0:64], AFT.Copy, scale=gk[:, 3 * w:3 * w + 1])
                    t2 = pot.tile([128, 64], F32, tag="t2")
                    nc.scalar.activation(t2, pe[:, 64:128], AFT.Copy, scale=gk[:, 3 * w + 1:3 * w + 2])
                    t3 = pot.tile([128, 64], F32, tag="t3")
                    nc.scalar.activation(t3, pe[:, 128:192], AFT.Copy, scale=gk[:, 3 * w + 2:3 * w + 3])
                    nc.gpsimd.tensor_tensor(t1, t1, t2, ALU.add)
                    nc.gpsimd.tensor_tensor(stg[:, 64 * w:64 * w + 64], t1, t3, ALU.add)
                    yield
                nc.gpsimd.dma_start(out=out_v[s], in_=stg.rearrange("p (w d) -> p w d", w=4))

            def advance(gen, n):
                if gen is None:
                    return None
                for _ in range(n):
                    try:
                        next(gen)
                    except StopIteration:
                        return None
                return gen

            pending = None
            for s in range(nsup):
                cstt = psup.tile([65, SB], BF16, tag="cstt")
                nc.sync.dma_start(out=cstt, in_=cstt_d[:, s * SB:(s + 1) * SB])
                cst2 = psup.tile([128, 256], BF16, tag="cst2")
                nc.sync.dma_start(out=cst2, in_=cst2_d[:, s * 256:(s + 1) * 256])
                mkf = psup.tile([104, 8 * MCH], BF16, tag="mkf")
                nc.gpsimd.dma_start(out=mkf, in_=mkf_d[s])
                agg3 = psup.tile([128, 768], BF16, tag="agg3")
                aggf2 = psup.tile([128, 256], BF16, tag="aggf2")

                # t8[b%128, 64*(b//128)+dout] = (cs@W1 + b_msg)[b, dout]
                pt = ppt.tile([128, 256], F32, tag="pt")
                for m in range(4):
                    nc.tensor.matmul(
                        pt[:, 64 * m:64 * m + 64],
                        cstt[:, 128 * m:128 * m + 128], w1b65,
                        start=True, stop=True,
                    )
                t8 = psup.tile([128, 256], BF16, tag="t8")
                nc.scalar.activation(t8, pt, AFT.Copy)

                for gg in range(SB // CB // GC):   # 2 groups per super
                    g = s * 2 + gg
                    mbl = pmb.tile([104, GC * NSF], F8E4, tag="mbl")
                    nc.gpsimd.dma_start(out=mbl, in_=m8_d[g])
                    nsT = pns.tile([128, GC * 832], BF16, tag="nsT")
                    assert use_dmat, "PE-transpose fallback not implemented"
                    nc.sync.dma_start_transpose(out=nsT, in_=nst_d[g])
                    for cc in range(GC):
                        emit_chunk(s, gg, cc, mbl, nsT, t8, mkf, agg3, aggf2)
                        pending = advance(pending, 2 if gg * GC + cc >= 5 else 1)
                advance(pending, 99)
                pending = emit_chain(s, cst2, agg3, aggf2)
            advance(pending, 99)
    return nc


# ---------------- host-side packing ----------------

def host_pack(inputs, core, Bc):
    b0 = core * Bc
    ns = np.asarray(inputs["neighbor_states"][b0:b0 + Bc], np.float32)
    cs = np.asarray(inputs["current_state"][b0:b0 + Bc], np.float32)
    tier = np.asarray(inputs["tier"][b0:b0 + Bc], np.int32)

    nch = Bc // CB
    ngr = nch // GC
    nsup = Bc // SB

    w_mask = []
    for t in (0, 2, 1):
        m = (tier == t)
        w_mask.append(m.astype(np.float32) / np.maximum(m.sum(-1, keepdims=True), 1.0))
    w0, w2m, w1 = w_mask  # local, dist, func
    wmean = np.full_like(w0, 1.0 / K)

    nsr = ns.reshape(nch, 16, 4, K, D)
    ns_tok = np.ascontiguousarray(nsr.transpose(0, 2, 3, 1, 4)).reshape(nch, 104, NSF)

    # mk[c, (g k), q, m, g'] nonzero when g' == g
    wm3 = np.stack([w0, w2m, wmean], 0).reshape(3, nch, 16, 4, K)  # [m,c,q,g,k]
    M = np.zeros((nch, 4, K, 16, 3, 4), np.float32)
    for g in range(4):
        M[:, g, :, :, :, g] = wm3[:, :, :, g, :].transpose(1, 3, 2, 0)
    mk = M.reshape(nch, 104, MKF)

    w1r = w1.reshape(nch, 16, 4, K)
    F = np.zeros((nch, 4, K, 16, 4), np.float32)
    for g in range(4):
        F[:, g, :, :, g] = w1r[:, :, g, :].transpose(0, 2, 1)
    fse = F.reshape(nch, 104, FSF)

    m8 = np.ascontiguousarray(
        ns_tok.astype(ml_dtypes.float8_e4m3).reshape(ngr, GC, 104, NSF).transpose(0, 2, 1, 3)
    ).reshape(ngr, 104, GC * NSF)
    mkf = np.concatenate([mk, fse], axis=2).astype(ml_dtypes.bfloat16)   # [nch,104,256]
    mkf = np.ascontiguousarray(
        mkf.reshape(nsup, 8, 104, MCH).transpose(0, 2, 1, 3)
    ).reshape(nsup, 104, 8 * MCH)

    # pair-major for dma-transpose: [c, 832=(u,(g,k)), 128=(par,d)]
    T = nsr.reshape(nch, 8, 2, 4, K, D).transpose(0, 1, 3, 4, 2, 5)
    nst = np.ascontiguousarray(T).reshape(ngr, GC * 832, 128).astype(ml_dtypes.bfloat16)

    cst = cs.T.astype(np.float32)                      # [64, Bc]
    cstt = np.concatenate([cst, np.ones((1, Bc), np.float32)], 0).astype(ml_dtypes.bfloat16)
    cst2 = np.ascontiguousarray(
        cst.reshape(D, nsup, 2, 256).transpose(2, 0, 1, 3)
    ).reshape(128, nsup * 256).astype(ml_dtypes.bfloat16)

    def blk(w):
        z = np.zeros((128, 128), np.float32)
        z[0:64, 0:64] = w
        z[64:128, 64:128] = w
        return z

    wl = np.asarray(inputs["W_local"], np.float32)
    wu = np.asarray(inputs["W_upd"], np.float32)
    wf = np.asarray(inputs["W_fcnf"], np.float32)
    wd = np.asarray(inputs["W_dcnf"], np.float32)
    wg1 = np.asarray(inputs["W_g1"], np.float32)
    wg2 = np.asarray(inputs["W_g2"], np.float32)
    wmsg = np.asarray(inputs["W_msg"], np.float32)

    wpack = np.zeros((128, WPC), np.float32)
    for k, w in [("wl1", wl[:D]), ("wlb", wl[D:]), ("wu1", wu[:D]), ("wub", wu[D:]),
                 ("wf", wf), ("wfs", 0.1 * wf), ("wda", wd[:D]), ("wdb", wd[D:]),
                 ("wds", 0.1 * wd[:D])]:
        wpack[:, WB[k]:WB[k] + 128] = blk(w)
    g1a = np.zeros((128, 64), np.float32)
    g1a[0:64, 0:32] = wg1[:D]; g1a[64:128, 32:64] = wg1[:D]
    g1b = np.zeros((128, 64), np.float32)
    g1b[0:64, 0:32] = wg1[D:]; g1b[64:128, 32:64] = wg1[D:]
    wpack[:, WG1A:WG1A + 64] = g1a
    wpack[:, WG1B:WG1B + 64] = g1b
    wpack[0:32, WG2:WG2 + 3] = wg2
    wpack[32:64, WG2 + 3:WG2 + 6] = wg2
    fpack = np.zeros((128, 8), np.float32)
    bg1 = np.asarray(inputs["b_g1"], np.float32)
    fpack[0:32, 0] = bg1; fpack[32:64, 0] = bg1
    fpack[64:67, 1] = np.asarray(inputs["b_g2"], np.float32)
    bl = np.asarray(inputs["b_local"], np.float32)
    bu = np.asarray(inputs["b_upd"], np.float32)
    bfc = np.asarray(inputs["b_fcnf"], np.float32)
    bdc = np.asarray(inputs["b_dcnf"], np.float32)
    fpack[0:64, 2] = bl; fpack[64:128, 2] = bl
    fpack[0:64, 3] = bu; fpack[64:128, 3] = bu
    fpack[0:64, 4] = bfc; fpack[64:128, 4] = bfc
    fpack[0:64, 5] = bdc; fpack[64:128, 5] = bdc

    bpack = np.zeros((128, BPC), np.float32)
    bpack[:, BW2:BW2 + 128] = blk(wmsg[D:])
    bpack[0:64, BW1B:BW1B + 64] = wmsg[:D]
    bpack[64, BW1B:BW1B + 64] = np.asarray(inputs["b_msg"], np.float32)
    ii = np.eye(64, dtype=np.float32)
    bpack[0:64, BI64:BI64 + 64] = ii; bpack[64:128, BI64:BI64 + 64] = ii
    bpack[0:64, BIP1:BIP1 + 64] = 0.1 * ii; bpack[64:128, BIP1:BIP1 + 64] = 0.1 * ii
    e3 = np.eye(3, dtype=np.float32)
    bpack[64:67, BEY3:BEY3 + 3] = e3
    tp = np.arange(104) // K            # t' // 26
    for ph in range(2):
        for u in range(8):
            for par in range(2):
                si = SELB + 104 * (((ph * 8 + u) * 2) + par)
                rows = 64 * ph + 8 * u + 4 * par + tp
                bpack[rows, si + np.arange(104)] = 1.0

    m = {
        "m8": m8,
        "mkf": mkf,
        "nst": nst,
        "cstt": cstt,
        "cst2": cst2,
        "wpack": wpack.astype(ml_dtypes.bfloat16),
        "fpack": fpack,
        "bpack": bpack.astype(ml_dtypes.bfloat16),
    }
    return {k: np.ascontiguousarray(v) for k, v in m.items()}


def _zb(inputs):
    return all(
        not np.any(np.asarray(inputs[k]))
        for k in ("b_local", "b_upd", "b_fcnf", "b_dcnf", "b_g2")
    )


_CACHE = {}


def _get_program(Bc, zb=True, use_dmat=True):
    key = (Bc, zb, use_dmat)
    if key not in _CACHE:
        nc = build_program(Bc, zb=zb, use_dmat=use_dmat)
        _split_waits(nc)
        _CACHE[key] = nc
    return _CACHE[key]


USE_DMAT = os.environ.get("K2_NO_DMAT", "") == ""


def run(inputs, trace=False):
    B = inputs["current_state"].shape[0]
    Bc = B // NCORES
    nc = _get_program(Bc, zb=_zb(inputs), use_dmat=USE_DMAT)
    in_maps = [host_pack(inputs, core, Bc) for core in range(NCORES)]
    res = run_bass_kernel_spmd(
        nc, in_maps, core_ids=list(range(NCORES)), trace=trace
    )
    out = np.concatenate([np.asarray(r["out"]).astype(np.float32) for r in res.results], axis=0)
    return out, res


def kernel(**inputs):
    out, _ = run(inputs)
    return out


# revision 3
# speedup vs baseline: 1.0814x; 1.0814x over previous
"""Trainium2 Bass kernel for nn_MoEConnectionProcessor (v2).

Math (per row b, D=64, K=26):
  masks from tier (0=local,1=func,2=dist)
  agg_l = masked_mean(ns, tier==0); h_local = tanh([cs,agg_l]@W_local)
  msg = relu(ns@W2 + (cs@W1 + b_msg)) per (b,k); agg_f = masked_mean(msg, tier==1)
  h = tanh([cs,agg_f]@W_upd); 3x Euler: h += .1*tanh(h@W_fcnf)
  agg_d = masked_mean(ns, tier==2); h_dist=cs; 3x: h += .1*tanh([h,agg_d]@W_dcnf)
  gates = softmax(relu([cs, mean_k ns]@W_g1)@W_g2); out = sum_k g_k * h_k

Strategy (data parallel, Bc=4096/core, all matmul operands bf16/fp8):
  - ns ships twice: fp8 token-major (aggregation matmuls contract the 104
    tokens of each 4-row tile against host-packed mask weights) and bf16
    pair-major, which one InstDmaTransposeAnt per 4-chunk group turns
    directly into the D-major msg lhsT (no PE transposes, no PSUM evac).
  - msg matmul covers 2 tiles per instruction via a block-diagonal
    [[W2,0],[0,W2]] rhs on the full 128-partition contraction; the
    (cs@W1+b_msg) broadcast rides per-chunk selector matmuls against a
    per-super b-major t8 tile (ones-row on csT carries b_msg).
  - the per-super expert/gating chain runs b-split [128,256] with
    block-diagonal weights so every matmul and activation uses all 128
    partitions; emission is software-pipelined (chain of super s
    interleaves the chunks of super s+1 to keep the in-order engine
    queues from stalling).
  - epilogue returns to b-major via accumulating identity matmuls that
    also fold the 0.1*sum Euler terms; gate softmax combines through
    per-partition scaled Copy activations; one out DMA per 512 rows.
"""

import os
import sys

sys.path.insert(0, "/opt/trn_rl_repo")

import numpy as np
import ml_dtypes

import concourse.bass as bass
import concourse.mybir as mybir
import concourse.tile as tile
from concourse.bass_utils import run_bass_kernel_spmd

F32 = mybir.dt.float32
F32R = mybir.dt.float32r
BF16 = mybir.dt.bfloat16
F8E4 = mybir.dt.float8e4

D = 64
K = 26
GH = 32
NCORES = 8
CB = 64            # b-rows per chunk (16 tiles of 4 rows / 104 tokens)
SB = 512           # b-rows per super (8 chunks)
GC = 4             # chunks per DMA group
AFT = mybir.ActivationFunctionType
ALU = mybir.AluOpType
AXL = mybir.AxisListType

# token-major ns ships fp8 (aggregation lhsT only); masks ride a per-super
# bf16 tensor: per chunk 192 mask cols (3 masks x 4 rows x 16 tiles) + 64 fse
NSF = 1024         # 16 tiles * 64
MKF = 192
FSF = 64
MCH = MKF + FSF    # 256 mask cols per chunk

# wpack (f32) column layout: 9 block-diagonal [128,128] weights,
# 2 [128,64] gating blocks, wg2blk [64,36], then bias columns.
WB = {
    "wl1": 0, "wlb": 128, "wu1": 256, "wub": 384, "wf": 512, "wfs": 640,
    "wda": 768, "wdb": 896, "wds": 1024,
}
WG1A = 1152
WG1B = 1216
WG2 = 1280         # [0:64, 1280:1316]
BG1 = 1316         # [0:64]
BG2 = 1317         # [64:100]  ([bg2;0...;bg2])
BL2 = 1318         # [128,1] [bl;bl]
BU2 = 1319
BFC2 = 1320
BDC2 = 1321
WPC = 1322

# bpack (bf16) column layout
BW2 = 0            # w2blk [128, 0:128]
BW1B = 128         # w1b65 [0:65, 128:192]
BI64 = 192         # identity stacked twice [128, 192:256]
BIP1 = 256         # 0.1*identity stacked twice [128, 256:320]
BEY3 = 320         # eye3 at rows 64:67, cols 320:323
SELB = 324         # 32 selector tiles [128,104]: idx=(ph*8+u)*2+par
BPC = 324 + 32 * 104


def r32(ap):
    return ap.bitcast(F32R)


def _split_waits(nc):
    """Walrus encodes at most one sync-wait command on most TPB instructions.
    Hoist extra on_wait entries into standalone single-wait EventSemaphore
    instructions placed immediately before, on the same engine queue."""
    for f in nc.m.functions:
        for blk in f.blocks:
            insts = list(blk.instructions)
            out = []
            changed = False
            for inst in insts:
                si = inst.sync_info
                ow = list(si.on_wait) if (si is not None and si.on_wait) else []
                if len(ow) > 1:
                    changed = True
                    for w in ow[:-1]:
                        out.append(
                            mybir.InstEventSemaphore(
                                name=nc.get_next_instruction_name(),
                                engine=inst.engine,
                                ins=[],
                                outs=[],
                                sync_info=mybir.SyncInfo(on_wait=[w], on_update=[]),
                            )
                        )
                    inst.sync_info = mybir.SyncInfo(
                        on_wait=[ow[-1]], on_update=list(si.on_update or [])
                    )
                out.append(inst)
            if changed:
                blk.instructions = out


def build_program(Bc, zb=True, use_dmat=True):
    assert Bc % SB == 0
    nsup = Bc // SB
    nch = Bc // CB
    ngr = nch // GC

    nc = bass.Bass(trn_type="TRN2", target_bir_lowering=False, debug=False)

    m8_d = nc.dram_tensor("m8", [ngr, 104, GC * NSF], F8E4, kind="ExternalInput").ap()
    mkf_d = nc.dram_tensor("mkf", [nsup, 104, 8 * MCH], BF16, kind="ExternalInput").ap()
    nst_d = nc.dram_tensor("nst", [ngr, GC * 832, 128], BF16, kind="ExternalInput").ap()
    cstt_d = nc.dram_tensor("cstt", [65, Bc], BF16, kind="ExternalInput").ap()
    cst2_d = nc.dram_tensor("cst2", [128, Bc // 2], BF16, kind="ExternalInput").ap()
    wpack_d = nc.dram_tensor("wpack", [128, WPC], BF16, kind="ExternalInput").ap()
    fpack_d = nc.dram_tensor("fpack", [128, 8], F32, kind="ExternalInput").ap()
    bpack_d = nc.dram_tensor("bpack", [128, BPC], BF16, kind="ExternalInput").ap()
    out_d = nc.dram_tensor("out", [Bc, D], BF16, kind="ExternalOutput").ap()

    out_v = out_d.rearrange("(s w p) d -> s p w d", p=128, w=4)

    with tile.TileContext(nc) as tc:
        with (
            tc.tile_pool(name="sing", bufs=1) as sing,
            tc.tile_pool(name="pns", bufs=4) as pns,
            tc.tile_pool(name="pmb", bufs=4) as pmb,
            tc.tile_pool(name="pmsgr", bufs=2) as pmsgr,
            tc.tile_pool(name="psup", bufs=2) as psup,
            tc.tile_pool(name="pch", bufs=2) as pch,
            tc.tile_pool(name="pot", bufs=4) as pot,
            tc.tile_pool(name="pstg", bufs=2) as pstg,
            tc.tile_pool(name="ppm", bufs=2, space="PSUM") as ppm,
            tc.tile_pool(name="ppag", bufs=2, space="PSUM") as ppag,
            tc.tile_pool(name="ppsc", bufs=2, space="PSUM") as ppsc,
            tc.tile_pool(name="ppe", bufs=2, space="PSUM") as ppe,
            tc.tile_pool(name="ppt", bufs=1, space="PSUM") as ppt,
        ):
            wpack = sing.tile([128, WPC], BF16, tag="wpack")
            fpack = sing.tile([128, 8], F32, tag="fpack")
            nc.sync.dma_start(out=fpack, in_=fpack_d)
            nc.sync.dma_start(out=wpack, in_=wpack_d)
            bpack = sing.tile([128, BPC], BF16, tag="bpack")
            nc.sync.dma_start(out=bpack, in_=bpack_d)

            w2blk = bpack[:, BW2:BW2 + 128]
            w1b65 = bpack[0:65, BW1B:BW1B + 64]
            i64a = bpack[0:64, BI64:BI64 + 64]
            i64b = bpack[64:128, BI64:BI64 + 64]
            ip1a = bpack[0:64, BIP1:BIP1 + 64]
            ip1b = bpack[64:128, BIP1:BIP1 + 64]
            ey3a = bpack[64:67, BEY3:BEY3 + 3]

            # startup observer matmuls so later matmuls carry <=1 wait
            pwu = ppsc.tile([128, 512], F32, tag="psc")
            nc.tensor.matmul(pwu[0:32, 0:32], wpack[0:32, 0:32], wpack[0:32, 0:32], start=True, stop=True)
            nc.tensor.matmul(pwu[0:32, 0:32], bpack[0:32, 0:32], bpack[0:32, 0:32], start=True, stop=True)

            def emit_chunk(s, gg, cc, mbl, nsT, t8, mkf, agg3, aggf2):
                h = gg * GC + cc          # chunk within super
                c0 = cc * NSF             # m8 col base for chunk
                m0 = (gg * GC + cc) * MCH # mkf col base for chunk
                n0 = cc * 832             # nsT col base
                hw_ = h % 4
                P0, P1 = (0, 64) if h < 4 else (64, 128)

                # ---- aggregation matmuls (only need mbl/mkf) ----
                pagg = ppag.tile([128, 256], F32, tag="pagg", bufs=1)
                for q in range(16):
                    nc.tensor.matmul(
                        pagg[P0:P1, 12 * q:12 * q + 12],
                        mbl[:, c0 + 64 * q: c0 + 64 * (q + 1)],
                        mkf[:, m0 + 12 * q: m0 + 12 * (q + 1)],
                        start=True, stop=True,
                    )

                # ---- msg matmuls: ns@W2blk + t-broadcast ----
                pmA = ppm.tile([104, 512], F32, tag="pm")
                pmB = ppm.tile([104, 512], F32, tag="pm")
                ph_ = h % 2
                t8s = t8[:, 64 * (h // 2):64 * (h // 2) + 64]
                for u in range(8):
                    pm = pmA if u < 4 else pmB
                    o0 = 128 * (u % 4)
                    nc.tensor.matmul(
                        pm[:, o0:o0 + 128],
                        nsT[:, n0 + 104 * u: n0 + 104 * (u + 1)],
                        w2blk, start=True, stop=False,
                    )
                    for par in range(2):
                        si = SELB + 104 * (((ph_ * 8 + u) * 2) + par)
                        nc.tensor.matmul(
                            pm[:, o0 + 64 * par: o0 + 64 * par + 64],
                            bpack[:, si:si + 104], t8s,
                            start=False, stop=(par == 1),
                        )
                msgr = pmsgr.tile([104, 1024], BF16, tag="msgr")
                nc.scalar.activation(msgr[:, 0:512], pmA, AFT.Relu)
                nc.vector.tensor_scalar(msgr[:, 512:1024], pmB, 0.0, None, ALU.max)

                for q in range(16):
                    nc.tensor.matmul(
                        pagg[P0:P1, 192 + 4 * q:196 + 4 * q],
                        msgr[:, 64 * q:64 * (q + 1)],
                        mkf[:, m0 + MKF + 4 * q: m0 + MKF + 4 * (q + 1)],
                        start=True, stop=True,
                    )
                nc.vector.tensor_copy(
                    agg3[P0:P1].rearrange("p (m bc) -> p m bc", m=3)
                    [:, :, 64 * hw_:64 * hw_ + 64]
                    .rearrange("p m (q g) -> p q m g", g=4),
                    pagg[P0:P1, 0:192].rearrange("p (q m g) -> p q m g", m=3, g=4),
                )
                nc.vector.tensor_copy(
                    aggf2[P0:P1, 64 * hw_:64 * hw_ + 64],
                    pagg[P0:P1, 192:256],
                )

            W = lambda k: wpack[:, WB[k]:WB[k] + 128]

            def emit_chain(s, cst2, agg3, aggf2):
                """Generator: chain+epilogue for super s; each step keeps the
                number of act-blocked PE matmuls under the 4-deep wait queue."""
                agl = agg3[:, 0:256]
                agd = agg3[:, 256:512]
                agm = agg3[:, 512:768]

                # step 1: everything that only needs the aggregates
                pzg = ppsc.tile([128, 512], F32, tag="psc")
                nc.tensor.matmul(pzg[0:64, 0:256], wpack[:, WG1A:WG1A + 64], cst2, start=True, stop=False)
                nc.tensor.matmul(pzg[0:64, 0:256], wpack[:, WG1B:WG1B + 64], agm, start=False, stop=True)
                g1r = pch.tile([64, 256], BF16, tag="g1r")
                nc.vector.tensor_scalar(
                    g1r, pzg[0:64, 0:256], fpack[0:64, 0:1], 0.0,
                    ALU.add, ALU.max,
                )
                ph = ppsc.tile([128, 512], F32, tag="psc")
                nc.tensor.matmul(ph[:, 0:256], W("wl1"), cst2, start=True, stop=False)
                nc.tensor.matmul(ph[:, 0:256], W("wlb"), agl, start=False, stop=True)
                nc.tensor.matmul(ph[:, 256:512], W("wu1"), cst2, start=True, stop=False)
                nc.tensor.matmul(ph[:, 256:512], W("wub"), aggf2, start=False, stop=True)
                hlu = pch.tile([128, 512], BF16, tag="hlu")
                if zb:
                    nc.scalar.activation(hlu, ph, AFT.Tanh)
                else:
                    nc.scalar.activation(hlu[:, 0:256], ph[:, 0:256], AFT.Tanh, bias=fpack[:, 2:3])
                    nc.scalar.activation(hlu[:, 256:512], ph[:, 256:512], AFT.Tanh, bias=fpack[:, 3:4])
                hl = hlu[:, 0:256]
                hu0 = hlu[:, 256:512]
                yield

                # step 2: logits (g1r ready by now) + tfd0 d-side (ready)
                pex = pzg[:, 384:400]
                exw = pot.tile([128, 12], F32, tag="exw")
                if zb:
                    for w in range(4):
                        lo = 32 * (w >= 2)
                        nc.tensor.matmul(
                            pex[:, 3 * w:3 * w + 3],
                            g1r[lo:lo + 32, 128 * (w % 2):128 * (w % 2) + 128],
                            wpack[lo:lo + 32, WG2 + 3 * (w >= 2):WG2 + 3 * (w >= 2) + 3],
                            start=True, stop=True,
                        )
                    nc.scalar.activation(exw, pex[:, 0:12], AFT.Exp)
                else:
                    nc.tensor.matmul(pzg[64:67, 0:256], wpack[0:64, WG2:WG2 + 3], g1r, start=True, stop=True)
                    nc.tensor.matmul(pzg[64:67, 256:512], wpack[0:64, WG2 + 3:WG2 + 6], g1r, start=True, stop=True)
                    expt = pch.tile([128, 512], BF16, tag="expt")
                    nc.scalar.activation(
                        expt[64:67], pzg[64:67, 0:512], AFT.Exp,
                        bias=fpack[64:67, 1:2],
                    )
                    for w in range(4):
                        eb = 256 * (w >= 2) + 128 * (w % 2)
                        nc.tensor.matmul(pex[:, 3 * w:3 * w + 3], expt[64:67, eb:eb + 128], ey3a, start=True, stop=True)
                    nc.vector.tensor_copy(exw, pex[:, 0:12])
                pz0 = ppsc.tile([128, 512], F32, tag="psc")
                nc.tensor.matmul(pz0[:, 256:512], W("wda"), cst2, start=True, stop=False)
                nc.tensor.matmul(pz0[:, 256:512], W("wdb"), agd, start=False, stop=True)
                yield

                # step 3: tfd0 f-side (waits hlu act) + act + softmax scalars
                nc.tensor.matmul(pz0[:, 0:256], W("wf"), hu0, start=True, stop=True)
                tfd0 = pch.tile([128, 512], BF16, tag="tfd0")
                if zb:
                    nc.scalar.activation(tfd0, pz0, AFT.Tanh)
                else:
                    nc.scalar.activation(tfd0[:, 0:256], pz0[:, 0:256], AFT.Tanh, bias=fpack[:, 4:5])
                    nc.scalar.activation(tfd0[:, 256:512], pz0[:, 256:512], AFT.Tanh, bias=fpack[:, 5:6])
                se = pot.tile([128, 4], F32, tag="se")
                nc.vector.tensor_reduce(
                    se.rearrange("p (w o) -> p w o", o=1),
                    exw.rearrange("p (w k) -> p w k", k=3),
                    AXL.X, ALU.add,
                )
                rc = pot.tile([128, 4], F32, tag="rc")
                nc.vector.reciprocal(rc, se)
                gk = pot.tile([128, 12], F32, tag="gk")
                for w in range(4):
                    nc.vector.tensor_scalar(
                        gk[:, 3 * w:3 * w + 3], exw[:, 3 * w:3 * w + 3],
                        rc[:, w:w + 1], None, ALU.mult,
                    )
                yield

                # steps 4-5: tfd1 (d-side ready mms first, tfd0-waiters later)
                pz1 = ppsc.tile([128, 512], F32, tag="psc")
                nc.tensor.matmul(pz1[:, 256:512], W("wda"), cst2, start=True, stop=False)
                nc.tensor.matmul(pz1[:, 256:512], W("wdb"), agd, start=False, stop=False)
                yield
                nc.tensor.matmul(pz1[:, 256:512], W("wds"), tfd0[:, 256:512], start=False, stop=True)
                nc.tensor.matmul(pz1[:, 0:256], W("wf"), hu0, start=True, stop=False)
                nc.tensor.matmul(pz1[:, 0:256], W("wfs"), tfd0[:, 0:256], start=False, stop=True)
                tfd1 = pch.tile([128, 512], BF16, tag="tfd1")
                if zb:
                    nc.scalar.activation(tfd1, pz1, AFT.Tanh)
                else:
                    nc.scalar.activation(tfd1[:, 0:256], pz1[:, 0:256], AFT.Tanh, bias=fpack[:, 4:5])
                    nc.scalar.activation(tfd1[:, 256:512], pz1[:, 256:512], AFT.Tanh, bias=fpack[:, 5:6])
                yield

                # steps 6-7: tfd2
                pz2 = ppsc.tile([128, 512], F32, tag="psc")
                nc.tensor.matmul(pz2[:, 256:512], W("wda"), cst2, start=True, stop=False)
                nc.tensor.matmul(pz2[:, 256:512], W("wdb"), agd, start=False, stop=False)
                nc.tensor.matmul(pz2[:, 256:512], W("wds"), tfd0[:, 256:512], start=False, stop=False)
                yield
                nc.tensor.matmul(pz2[:, 256:512], W("wds"), tfd1[:, 256:512], start=False, stop=True)
                nc.tensor.matmul(pz2[:, 0:256], W("wf"), hu0, start=True, stop=False)
                nc.tensor.matmul(pz2[:, 0:256], W("wfs"), tfd0[:, 0:256], start=False, stop=False)
                nc.tensor.matmul(pz2[:, 0:256], W("wfs"), tfd1[:, 0:256], start=False, stop=True)
                tfd2 = pch.tile([128, 512], BF16, tag="tfd2")
                if zb:
                    nc.scalar.activation(tfd2, pz2, AFT.Tanh)
                else:
                    nc.scalar.activation(tfd2[:, 0:256], pz2[:, 0:256], AFT.Tanh, bias=fpack[:, 4:5])
                    nc.scalar.activation(tfd2[:, 256:512], pz2[:, 256:512], AFT.Tanh, bias=fpack[:, 5:6])
                tfds = [tfd0, tfd1, tfd2]
                yield

                stg = pstg.tile([128, 256], BF16, tag="stg")
                for w in range(4):
                    hi = w >= 2
                    R0, R1 = (64, 128) if hi else (0, 64)
                    b0 = 128 * (w % 2)
                    i64 = i64b if hi else i64a
                    ip1 = ip1b if hi else ip1a
                    sl = slice(b0, b0 + 128)
                    pe = ppe.tile([128, 256], F32, tag="pe")
                    nc.tensor.matmul(pe[:, 0:64], hl[R0:R1, sl], i64, start=True, stop=True)
                    nc.tensor.matmul(pe[:, 64:128], hu0[R0:R1, sl], i64, start=True, stop=False)
                    nc.tensor.matmul(pe[:, 64:128], tfds[0][R0:R1, sl], ip1, start=False, stop=False)
                    nc.tensor.matmul(pe[:, 64:128], tfds[1][R0:R1, sl], ip1, start=False, stop=False)
                    nc.tensor.matmul(pe[:, 64:128], tfds[2][R0:R1, sl], ip1, start=False, stop=True)
                    nc.tensor.matmul(pe[:, 128:192], cst2[R0:R1, sl], i64, start=True, stop=False)
                    nc.tensor.matmul(pe[:, 128:192], tfds[0][R0:R1, 256 + b0:256 + b0 + 128], ip1, start=False, stop=False)
                    nc.tensor.matmul(pe[:, 128:192], tfds[1][R0:R1, 256 + b0:256 + b0 + 128], ip1, start=False, stop=False)
                    nc.tensor.matmul(pe[:, 128:192], tfds[2][R0:R1, 256 + b0:256 + b0 + 128], ip1, start=False, stop=True)
                    t1 = pot.tile([128, 64], F32, tag="t1")
                    nc.scalar.activation(t1, pe[:, 0:64], AFT.Copy, scale=gk[:, 3 * w:3 * w + 1])
                    t2 = pot.tile([128, 64], F32, tag="t2")
                    nc.scalar.activation(t2, pe[:, 64:128], AFT.Copy, scale=gk[:, 3 * w + 1:3 * w + 2])
                    t3 = pot.tile([128, 64], F32, tag="t3")
                    nc.scalar.activation(t3, pe[:, 128:192], AFT.Copy, scale=gk[:, 3 * w + 2:3 * w + 3])
                    nc.gpsimd.tensor_tensor(t1, t1, t2, ALU.add)
                    nc.gpsimd.tensor_tensor(stg[:, 64 * w:64 * w + 64], t1, t3, ALU.add)
                    yield
                nc.gpsimd.dma_start(out=out_v[s], in_=stg.rearrange("p (w d) -> p w d", w=4))

            def advance(gen, n):
                if gen is None:
                    return None
                for _ in range(n):
                    try:
                        next(gen)
                    except StopIteration:
                        return None
                return gen

            pending = None
            for s in range(nsup):
                cstt = psup.tile([65, SB], BF16, tag="cstt")
                nc.sync.dma_start(out=cstt, in_=cstt_d[:, s * SB:(s + 1) * SB])
                cst2 = psup.tile([128, 256], BF16, tag="cst2")
                nc.sync.dma_start(out=cst2, in_=cst2_d[:, s * 256:(s + 1) * 256])
                mkf = psup.tile([104, 8 * MCH], BF16, tag="mkf")
                nc.gpsimd.dma_start(out=mkf, in_=mkf_d[s])
                agg3 = psup.tile([128, 768], BF16, tag="agg3")
                aggf2 = psup.tile([128, 256], BF16, tag="aggf2")

                # t8[b%128, 64*(b//128)+dout] = (cs@W1 + b_msg)[b, dout]
                pt = ppt.tile([128, 256], F32, tag="pt")
                for m in range(4):
                    nc.tensor.matmul(
                        pt[:, 64 * m:64 * m + 64],
                        cstt[:, 128 * m:128 * m + 128], w1b65,
                        start=True, stop=True,
                    )
                t8 = psup.tile([128, 256], BF16, tag="t8")
                nc.scalar.activation(t8, pt, AFT.Copy)

                for gg in range(SB // CB // GC):   # 2 groups per super
                    g = s * 2 + gg
                    mbl = pmb.tile([104, GC * NSF], F8E4, tag="mbl")
                    nc.gpsimd.dma_start(out=mbl, in_=m8_d[g])
                    nsT = pns.tile([128, GC * 832], BF16, tag="nsT")
                    assert use_dmat, "PE-transpose fallback not implemented"
                    nc.sync.dma_start_transpose(out=nsT, in_=nst_d[g])
                    for cc in range(GC):
                        emit_chunk(s, gg, cc, mbl, nsT, t8, mkf, agg3, aggf2)
                        pending = advance(pending, 2 if gg * GC + cc >= 5 else 1)
                advance(pending, 99)
                pending = emit_chain(s, cst2, agg3, aggf2)
            advance(pending, 99)
    return nc


# ---------------- host-side packing ----------------

def host_pack(inputs, core, Bc):
    b0 = core * Bc
    ns = np.asarray(inputs["neighbor_states"][b0:b0 + Bc], np.float32)
    cs = np.asarray(inputs["current_state"][b0:b0 + Bc], np.float32)
    tier = np.asarray(inputs["tier"][b0:b0 + Bc], np.int32)

    nch = Bc // CB
    ngr = nch // GC
    nsup = Bc // SB

    w_mask = []
    for t in (0, 2, 1):
        m = (tier == t)
        w_mask.append(m.astype(np.float32) / np.maximum(m.sum(-1, keepdims=True), 1.0))
    w0, w2m, w1 = w_mask  # local, dist, func
    wmean = np.full_like(w0, 1.0 / K)

    nsr = ns.reshape(nch, 16, 4, K, D)
    ns_tok = np.ascontiguousarray(nsr.transpose(0, 2, 3, 1, 4)).reshape(nch, 104, NSF)

    # mk[c, (g k), q, m, g'] nonzero when g' == g
    wm3 = np.stack([w0, w2m, wmean], 0).reshape(3, nch, 16, 4, K)  # [m,c,q,g,k]
    M = np.zeros((nch, 4, K, 16, 3, 4), np.float32)
    for g in range(4):
        M[:, g, :, :, :, g] = wm3[:, :, :, g, :].transpose(1, 3, 2, 0)
    mk = M.reshape(nch, 104, MKF)

    w1r = w1.reshape(nch, 16, 4, K)
    F = np.zeros((nch, 4, K, 16, 4), np.float32)
    for g in range(4):
        F[:, g, :, :, g] = w1r[:, :, g, :].transpose(0, 2, 1)
    fse = F.reshape(nch, 104, FSF)

    m8 = np.ascontiguousarray(
        ns_tok.astype(ml_dtypes.float8_e4m3).reshape(ngr, GC, 104, NSF).transpose(0, 2, 1, 3)
    ).reshape(ngr, 104, GC * NSF)
    mkf = np.concatenate([mk, fse], axis=2).astype(ml_dtypes.bfloat16)   # [nch,104,256]
    mkf = np.ascontiguousarray(
        mkf.reshape(nsup, 8, 104, MCH).transpose(0, 2, 1, 3)
    ).reshape(nsup, 104, 8 * MCH)

    # pair-major for dma-transpose: [c, 832=(u,(g,k)), 128=(par,d)]
    T = nsr.reshape(nch, 8, 2, 4, K, D).transpose(0, 1, 3, 4, 2, 5)
    nst = np.ascontiguousarray(T).reshape(ngr, GC * 832, 128).astype(ml_dtypes.bfloat16)

    cst = cs.T.astype(np.float32)                      # [64, Bc]
    cstt = np.concatenate([cst, np.ones((1, Bc), np.float32)], 0).astype(ml_dtypes.bfloat16)
    cst2 = np.ascontiguousarray(
        cst.reshape(D, nsup, 2, 256).transpose(2, 0, 1, 3)
    ).reshape(128, nsup * 256).astype(ml_dtypes.bfloat16)

    def blk(w):
        z = np.zeros((128, 128), np.float32)
        z[0:64, 0:64] = w
        z[64:128, 64:128] = w
        return z

    wl = np.asarray(inputs["W_local"], np.float32)
    wu = np.asarray(inputs["W_upd"], np.float32)
    wf = np.asarray(inputs["W_fcnf"], np.float32)
    wd = np.asarray(inputs["W_dcnf"], np.float32)
    wg1 = np.asarray(inputs["W_g1"], np.float32)
    wg2 = np.asarray(inputs["W_g2"], np.float32)
    wmsg = np.asarray(inputs["W_msg"], np.float32)

    wpack = np.zeros((128, WPC), np.float32)
    for k, w in [("wl1", wl[:D]), ("wlb", wl[D:]), ("wu1", wu[:D]), ("wub", wu[D:]),
                 ("wf", wf), ("wfs", 0.1 * wf), ("wda", wd[:D]), ("wdb", wd[D:]),
                 ("wds", 0.1 * wd[:D])]:
        wpack[:, WB[k]:WB[k] + 128] = blk(w)
    g1a = np.zeros((128, 64), np.float32)
    g1a[0:64, 0:32] = wg1[:D]; g1a[64:128, 32:64] = wg1[:D]
    g1b = np.zeros((128, 64), np.float32)
    g1b[0:64, 0:32] = wg1[D:]; g1b[64:128, 32:64] = wg1[D:]
    wpack[:, WG1A:WG1A + 64] = g1a
    wpack[:, WG1B:WG1B + 64] = g1b
    wpack[0:32, WG2:WG2 + 3] = wg2
    wpack[32:64, WG2 + 3:WG2 + 6] = wg2
    fpack = np.zeros((128, 8), np.float32)
    bg1 = np.asarray(inputs["b_g1"], np.float32)
    fpack[0:32, 0] = bg1; fpack[32:64, 0] = bg1
    fpack[64:67, 1] = np.asarray(inputs["b_g2"], np.float32)
    bl = np.asarray(inputs["b_local"], np.float32)
    bu = np.asarray(inputs["b_upd"], np.float32)
    bfc = np.asarray(inputs["b_fcnf"], np.float32)
    bdc = np.asarray(inputs["b_dcnf"], np.float32)
    fpack[0:64, 2] = bl; fpack[64:128, 2] = bl
    fpack[0:64, 3] = bu; fpack[64:128, 3] = bu
    fpack[0:64, 4] = bfc; fpack[64:128, 4] = bfc
    fpack[0:64, 5] = bdc; fpack[64:128, 5] = bdc

    bpack = np.zeros((128, BPC), np.float32)
    bpack[:, BW2:BW2 + 128] = blk(wmsg[D:])
    bpack[0:64, BW1B:BW1B + 64] = wmsg[:D]
    bpack[64, BW1B:BW1B + 64] = np.asarray(inputs["b_msg"], np.float32)
    ii = np.eye(64, dtype=np.float32)
    bpack[0:64, BI64:BI64 + 64] = ii; bpack[64:128, BI64:BI64 + 64] = ii
    bpack[0:64, BIP1:BIP1 + 64] = 0.1 * ii; bpack[64:128, BIP1:BIP1 + 64] = 0.1 * ii
    e3 = np.eye(3, dtype=np.float32)
    bpack[64:67, BEY3:BEY3 + 3] = e3
    tp = np.arange(104) // K            # t' // 26
    for ph in range(2):
        for u in range(8):
            for par in range(2):
                si = SELB + 104 * (((ph * 8 + u) * 2) + par)
                rows = 64 * ph + 8 * u + 4 * par + tp
                bpack[rows, si + np.arange(104)] = 1.0

    m = {
        "m8": m8,
        "mkf": mkf,
        "nst": nst,
        "cstt": cstt,
        "cst2": cst2,
        "wpack": wpack.astype(ml_dtypes.bfloat16),
        "fpack": fpack,
        "bpack": bpack.astype(ml_dtypes.bfloat16),
    }
    return {k: np.ascontiguousarray(v) for k, v in m.items()}


def _zb(inputs):
    return all(
        not np.any(np.asarray(inputs[k]))
        for k in ("b_local", "b_upd", "b_fcnf", "b_dcnf", "b_g2")
    )


_CACHE = {}


def _get_program(Bc, zb=True, use_dmat=True):
    key = (Bc, zb, use_dmat)
    if key not in _CACHE:
        nc = build_program(Bc, zb=zb, use_dmat=use_dmat)
        _split_waits(nc)
        _CACHE[key] = nc
    return _CACHE[key]


USE_DMAT = os.environ.get("K2_NO_DMAT", "") == ""


def run(inputs, trace=False):
    B = inputs["current_state"].shape[0]
    Bc = B // NCORES
    nc = _get_program(Bc, zb=_zb(inputs), use_dmat=USE_DMAT)
    in_maps = [host_pack(inputs, core, Bc) for core in range(NCORES)]
    res = run_bass_kernel_spmd(
        nc, in_maps, core_ids=list(range(NCORES)), trace=trace
    )
    out = np.concatenate([np.asarray(r["out"]).astype(np.float32) for r in res.results], axis=0)
    return out, res


def kernel(**inputs):
    out, _ = run(inputs)
    return out


# revision 4
# speedup vs baseline: 1.0982x; 1.0155x over previous
"""Trainium2 Bass kernel for nn_MoEConnectionProcessor (v2).

Math (per row b, D=64, K=26):
  masks from tier (0=local,1=func,2=dist)
  agg_l = masked_mean(ns, tier==0); h_local = tanh([cs,agg_l]@W_local)
  msg = relu(ns@W2 + (cs@W1 + b_msg)) per (b,k); agg_f = masked_mean(msg, tier==1)
  h = tanh([cs,agg_f]@W_upd); 3x Euler: h += .1*tanh(h@W_fcnf)
  agg_d = masked_mean(ns, tier==2); h_dist=cs; 3x: h += .1*tanh([h,agg_d]@W_dcnf)
  gates = softmax(relu([cs, mean_k ns]@W_g1)@W_g2); out = sum_k g_k * h_k

Strategy (data parallel, Bc=4096/core, all matmul operands bf16/fp8):
  - ns ships twice: fp8 token-major (aggregation matmuls contract the 104
    tokens of each 4-row tile against host-packed mask weights) and bf16
    pair-major, which one InstDmaTransposeAnt per 4-chunk group turns
    directly into the D-major msg lhsT (no PE transposes, no PSUM evac).
  - msg matmul covers 2 tiles per instruction via a block-diagonal
    [[W2,0],[0,W2]] rhs on the full 128-partition contraction; the
    (cs@W1+b_msg) broadcast rides per-chunk selector matmuls against a
    per-super b-major t8 tile (ones-row on csT carries b_msg).
  - the per-super expert/gating chain runs b-split [128,256] with
    block-diagonal weights so every matmul and activation uses all 128
    partitions; emission is software-pipelined (chain of super s
    interleaves the chunks of super s+1 to keep the in-order engine
    queues from stalling).
  - epilogue returns to b-major via accumulating identity matmuls that
    also fold the 0.1*sum Euler terms; gate softmax combines through
    per-partition scaled Copy activations; one out DMA per 512 rows.
"""

import os
import sys

sys.path.insert(0, "/opt/trn_rl_repo")

import numpy as np
import ml_dtypes

import concourse.bass as bass
import concourse.mybir as mybir
import concourse.tile as tile
from concourse.bass_utils import run_bass_kernel_spmd

F32 = mybir.dt.float32
F32R = mybir.dt.float32r
BF16 = mybir.dt.bfloat16
F8E4 = mybir.dt.float8e4

D = 64
K = 26
GH = 32
NCORES = 8
CB = 64            # b-rows per chunk (16 tiles of 4 rows / 104 tokens)
SB = 512           # b-rows per super (8 chunks)
GC = 4             # chunks per DMA group
AFT = mybir.ActivationFunctionType
ALU = mybir.AluOpType
AXL = mybir.AxisListType

# token-major ns ships fp8 (aggregation lhsT only); masks ride a per-super
# bf16 tensor: per chunk 192 mask cols (3 masks x 4 rows x 16 tiles) + 64 fse
NSF = 1024         # 16 tiles * 64
MKF = 192
FSF = 64
MCH = MKF + FSF    # 256 mask cols per chunk

# wpack (f32) column layout: 9 block-diagonal [128,128] weights,
# 2 [128,64] gating blocks, wg2blk [64,36], then bias columns.
WB = {
    "wl1": 0, "wlb": 128, "wu1": 256, "wub": 384, "wf": 512, "wfs": 640,
    "wda": 768, "wdb": 896, "wds": 1024,
}
WG1A = 1152
WG1B = 1216
WG2 = 1280         # [0:64, 1280:1316]
BG1 = 1316         # [0:64]
BG2 = 1317         # [64:100]  ([bg2;0...;bg2])
BL2 = 1318         # [128,1] [bl;bl]
BU2 = 1319
BFC2 = 1320
BDC2 = 1321
WPC = 1322

# bpack (bf16) column layout
BW2 = 0            # w2blk [128, 0:128]
BW1B = 128         # w1b65 [0:65, 128:192]
BI64 = 192         # identity stacked twice [128, 192:256]
BIP1 = 256         # 0.1*identity stacked twice [128, 256:320]
BEY3 = 320         # eye3 at rows 64:67, cols 320:323
SELB = 324         # 32 selector tiles [128,104]: idx=(ph*8+u)*2+par
BPC = 324 + 32 * 104


def r32(ap):
    return ap.bitcast(F32R)


def _split_waits(nc):
    """Walrus encodes at most one sync-wait command on most TPB instructions.
    Hoist extra on_wait entries into standalone single-wait EventSemaphore
    instructions placed immediately before, on the same engine queue."""
    for f in nc.m.functions:
        for blk in f.blocks:
            insts = list(blk.instructions)
            out = []
            changed = False
            for inst in insts:
                si = inst.sync_info
                ow = list(si.on_wait) if (si is not None and si.on_wait) else []
                if len(ow) > 1:
                    changed = True
                    for w in ow[:-1]:
                        out.append(
                            mybir.InstEventSemaphore(
                                name=nc.get_next_instruction_name(),
                                engine=inst.engine,
                                ins=[],
                                outs=[],
                                sync_info=mybir.SyncInfo(on_wait=[w], on_update=[]),
                            )
                        )
                    inst.sync_info = mybir.SyncInfo(
                        on_wait=[ow[-1]], on_update=list(si.on_update or [])
                    )
                out.append(inst)
            if changed:
                blk.instructions = out


def build_program(Bc, zb=True, use_dmat=True):
    assert Bc % SB == 0
    nsup = Bc // SB
    nch = Bc // CB
    ngr = nch // GC

    nc = bass.Bass(trn_type="TRN2", target_bir_lowering=False, debug=False)

    m8_d = nc.dram_tensor("m8", [ngr, 104, GC * NSF], F8E4, kind="ExternalInput").ap()
    mkf_d = nc.dram_tensor("mkf", [nsup, 104, 8 * MCH], BF16, kind="ExternalInput").ap()
    nst_d = nc.dram_tensor("nst", [ngr, GC * 832, 128], BF16, kind="ExternalInput").ap()
    cstt_d = nc.dram_tensor("cstt", [65, Bc], BF16, kind="ExternalInput").ap()
    cst2_d = nc.dram_tensor("cst2", [128, Bc // 2], BF16, kind="ExternalInput").ap()
    wpack_d = nc.dram_tensor("wpack", [128, WPC], BF16, kind="ExternalInput").ap()
    fpack_d = nc.dram_tensor("fpack", [128, 8], F32, kind="ExternalInput").ap()
    bpack_d = nc.dram_tensor("bpack", [128, BPC], BF16, kind="ExternalInput").ap()
    out_d = nc.dram_tensor("out", [Bc, D], BF16, kind="ExternalOutput").ap()

    out_v = out_d.rearrange("(s w p) d -> s p w d", p=128, w=4)

    with tile.TileContext(nc) as tc:
        with (
            tc.tile_pool(name="sing", bufs=1) as sing,
            tc.tile_pool(name="pns", bufs=4) as pns,
            tc.tile_pool(name="pmb", bufs=4) as pmb,
            tc.tile_pool(name="pmsgr", bufs=2) as pmsgr,
            tc.tile_pool(name="psup", bufs=2) as psup,
            tc.tile_pool(name="pch", bufs=2) as pch,
            tc.tile_pool(name="pot", bufs=4) as pot,
            tc.tile_pool(name="pstg", bufs=2) as pstg,
            tc.tile_pool(name="ppm", bufs=2, space="PSUM") as ppm,
            tc.tile_pool(name="ppag", bufs=2, space="PSUM") as ppag,
            tc.tile_pool(name="ppsc", bufs=2, space="PSUM") as ppsc,
            tc.tile_pool(name="ppe", bufs=2, space="PSUM") as ppe,
            tc.tile_pool(name="ppt", bufs=1, space="PSUM") as ppt,
        ):
            wpack = sing.tile([128, WPC], BF16, tag="wpack")
            fpack = sing.tile([128, 8], F32, tag="fpack")
            nc.scalar.dma_start(out=fpack, in_=fpack_d)
            nc.scalar.dma_start(out=wpack, in_=wpack_d)
            bpack = sing.tile([128, BPC], BF16, tag="bpack")
            nc.scalar.dma_start(out=bpack, in_=bpack_d)

            w2blk = bpack[:, BW2:BW2 + 128]
            w1b65 = bpack[0:65, BW1B:BW1B + 64]
            i64a = bpack[0:64, BI64:BI64 + 64]
            i64b = bpack[64:128, BI64:BI64 + 64]
            ip1a = bpack[0:64, BIP1:BIP1 + 64]
            ip1b = bpack[64:128, BIP1:BIP1 + 64]
            ey3a = bpack[64:67, BEY3:BEY3 + 3]

            # startup observer matmuls so later matmuls carry <=1 wait
            pwu = ppsc.tile([128, 512], F32, tag="psc")
            nc.tensor.matmul(pwu[0:32, 0:32], wpack[0:32, 0:32], wpack[0:32, 0:32], start=True, stop=True)
            nc.tensor.matmul(pwu[0:32, 0:32], bpack[0:32, 0:32], bpack[0:32, 0:32], start=True, stop=True)

            def emit_chunk(s, gg, cc, mbl, nsT, t8, mkf, agg3, aggf2):
                h = gg * GC + cc          # chunk within super
                c0 = cc * NSF             # m8 col base for chunk
                m0 = (gg * GC + cc) * MCH # mkf col base for chunk
                n0 = cc * 832             # nsT col base
                hw_ = h % 4
                P0, P1 = (0, 64) if h < 4 else (64, 128)

                # ---- aggregation matmuls (only need mbl/mkf) ----
                pagg = ppag.tile([128, 256], F32, tag="pagg", bufs=1)
                for q in range(16):
                    nc.tensor.matmul(
                        pagg[P0:P1, 12 * q:12 * q + 12],
                        mbl[:, c0 + 64 * q: c0 + 64 * (q + 1)],
                        mkf[:, m0 + 12 * q: m0 + 12 * (q + 1)],
                        start=True, stop=True,
                    )

                # ---- msg matmuls: ns@W2blk + t-broadcast ----
                pmA = ppm.tile([104, 512], F32, tag="pm")
                pmB = ppm.tile([104, 512], F32, tag="pm")
                ph_ = h % 2
                t8s = t8[:, 64 * (h // 2):64 * (h // 2) + 64]
                for u in range(8):
                    pm = pmA if u < 4 else pmB
                    o0 = 128 * (u % 4)
                    nc.tensor.matmul(
                        pm[:, o0:o0 + 128],
                        nsT[:, n0 + 104 * u: n0 + 104 * (u + 1)],
                        w2blk, start=True, stop=False,
                    )
                    for par in range(2):
                        si = SELB + 104 * (((ph_ * 8 + u) * 2) + par)
                        nc.tensor.matmul(
                            pm[:, o0 + 64 * par: o0 + 64 * par + 64],
                            bpack[:, si:si + 104], t8s,
                            start=False, stop=(par == 1),
                        )
                msgr = pmsgr.tile([104, 1024], BF16, tag="msgr")
                nc.scalar.activation(msgr[:, 0:512], pmA, AFT.Relu)
                nc.vector.tensor_scalar(msgr[:, 512:1024], pmB, 0.0, None, ALU.max)

                for q in range(16):
                    nc.tensor.matmul(
                        pagg[P0:P1, 192 + 4 * q:196 + 4 * q],
                        msgr[:, 64 * q:64 * (q + 1)],
                        mkf[:, m0 + MKF + 4 * q: m0 + MKF + 4 * (q + 1)],
                        start=True, stop=True,
                    )
                nc.vector.tensor_copy(
                    agg3[P0:P1].rearrange("p (m bc) -> p m bc", m=3)
                    [:, :, 64 * hw_:64 * hw_ + 64]
                    .rearrange("p m (q g) -> p q m g", g=4),
                    pagg[P0:P1, 0:192].rearrange("p (q m g) -> p q m g", m=3, g=4),
                )
                nc.vector.tensor_copy(
                    aggf2[P0:P1, 64 * hw_:64 * hw_ + 64],
                    pagg[P0:P1, 192:256],
                )

            W = lambda k: wpack[:, WB[k]:WB[k] + 128]

            def emit_chain(s, cst2, agg3, aggf2):
                """Generator: chain+epilogue for super s; each step keeps the
                number of act-blocked PE matmuls under the 4-deep wait queue."""
                agl = agg3[:, 0:256]
                agd = agg3[:, 256:512]
                agm = agg3[:, 512:768]

                # step 1: everything that only needs the aggregates
                pzg = ppsc.tile([128, 512], F32, tag="psc")
                nc.tensor.matmul(pzg[0:64, 0:256], wpack[:, WG1A:WG1A + 64], cst2, start=True, stop=False)
                nc.tensor.matmul(pzg[0:64, 0:256], wpack[:, WG1B:WG1B + 64], agm, start=False, stop=True)
                g1r = pch.tile([64, 256], BF16, tag="g1r")
                nc.vector.tensor_scalar(
                    g1r, pzg[0:64, 0:256], fpack[0:64, 0:1], 0.0,
                    ALU.add, ALU.max,
                )
                ph = ppsc.tile([128, 512], F32, tag="psc")
                nc.tensor.matmul(ph[:, 0:256], W("wl1"), cst2, start=True, stop=False)
                nc.tensor.matmul(ph[:, 0:256], W("wlb"), agl, start=False, stop=True)
                nc.tensor.matmul(ph[:, 256:512], W("wu1"), cst2, start=True, stop=False)
                nc.tensor.matmul(ph[:, 256:512], W("wub"), aggf2, start=False, stop=True)
                hlu = pch.tile([128, 512], BF16, tag="hlu")
                if zb:
                    nc.scalar.activation(hlu, ph, AFT.Tanh)
                else:
                    nc.scalar.activation(hlu[:, 0:256], ph[:, 0:256], AFT.Tanh, bias=fpack[:, 2:3])
                    nc.scalar.activation(hlu[:, 256:512], ph[:, 256:512], AFT.Tanh, bias=fpack[:, 3:4])
                hl = hlu[:, 0:256]
                hu0 = hlu[:, 256:512]
                yield

                # step 2: logits (g1r ready by now) + tfd0 d-side (ready)
                pex = pzg[:, 384:400]
                exw = pot.tile([128, 12], F32, tag="exw")
                if zb:
                    for w in range(4):
                        lo = 32 * (w >= 2)
                        nc.tensor.matmul(
                            pex[:, 3 * w:3 * w + 3],
                            g1r[lo:lo + 32, 128 * (w % 2):128 * (w % 2) + 128],
                            wpack[lo:lo + 32, WG2 + 3 * (w >= 2):WG2 + 3 * (w >= 2) + 3],
                            start=True, stop=True,
                        )
                    nc.scalar.activation(exw, pex[:, 0:12], AFT.Exp)
                else:
                    nc.tensor.matmul(pzg[64:67, 0:256], wpack[0:64, WG2:WG2 + 3], g1r, start=True, stop=True)
                    nc.tensor.matmul(pzg[64:67, 256:512], wpack[0:64, WG2 + 3:WG2 + 6], g1r, start=True, stop=True)
                    expt = pch.tile([128, 512], BF16, tag="expt")
                    nc.scalar.activation(
                        expt[64:67], pzg[64:67, 0:512], AFT.Exp,
                        bias=fpack[64:67, 1:2],
                    )
                    for w in range(4):
                        eb = 256 * (w >= 2) + 128 * (w % 2)
                        nc.tensor.matmul(pex[:, 3 * w:3 * w + 3], expt[64:67, eb:eb + 128], ey3a, start=True, stop=True)
                    nc.vector.tensor_copy(exw, pex[:, 0:12])
                pz0 = ppsc.tile([128, 512], F32, tag="psc")
                nc.tensor.matmul(pz0[:, 256:512], W("wda"), cst2, start=True, stop=False)
                nc.tensor.matmul(pz0[:, 256:512], W("wdb"), agd, start=False, stop=True)
                yield

                # step 3: tfd0 f-side (waits hlu act) + act + softmax scalars
                nc.tensor.matmul(pz0[:, 0:256], W("wf"), hu0, start=True, stop=True)
                tfd0 = pch.tile([128, 512], BF16, tag="tfd0")
                if zb:
                    nc.scalar.activation(tfd0, pz0, AFT.Tanh)
                else:
                    nc.scalar.activation(tfd0[:, 0:256], pz0[:, 0:256], AFT.Tanh, bias=fpack[:, 4:5])
                    nc.scalar.activation(tfd0[:, 256:512], pz0[:, 256:512], AFT.Tanh, bias=fpack[:, 5:6])
                se = pot.tile([128, 4], F32, tag="se")
                nc.vector.tensor_reduce(
                    se.rearrange("p (w o) -> p w o", o=1),
                    exw.rearrange("p (w k) -> p w k", k=3),
                    AXL.X, ALU.add,
                )
                rc = pot.tile([128, 4], F32, tag="rc")
                nc.vector.reciprocal(rc, se)
                gk = pot.tile([128, 12], F32, tag="gk")
                for w in range(4):
                    nc.vector.tensor_scalar(
                        gk[:, 3 * w:3 * w + 3], exw[:, 3 * w:3 * w + 3],
                        rc[:, w:w + 1], None, ALU.mult,
                    )
                yield

                # steps 4-5: tfd1 (d-side ready mms first, tfd0-waiters later)
                pz1 = ppsc.tile([128, 512], F32, tag="psc")
                nc.tensor.matmul(pz1[:, 256:512], W("wda"), cst2, start=True, stop=False)
                nc.tensor.matmul(pz1[:, 256:512], W("wdb"), agd, start=False, stop=False)
                yield
                nc.tensor.matmul(pz1[:, 256:512], W("wds"), tfd0[:, 256:512], start=False, stop=True)
                nc.tensor.matmul(pz1[:, 0:256], W("wf"), hu0, start=True, stop=False)
                nc.tensor.matmul(pz1[:, 0:256], W("wfs"), tfd0[:, 0:256], start=False, stop=True)
                tfd1 = pch.tile([128, 512], BF16, tag="tfd1")
                if zb:
                    nc.scalar.activation(tfd1, pz1, AFT.Tanh)
                else:
                    nc.scalar.activation(tfd1[:, 0:256], pz1[:, 0:256], AFT.Tanh, bias=fpack[:, 4:5])
                    nc.scalar.activation(tfd1[:, 256:512], pz1[:, 256:512], AFT.Tanh, bias=fpack[:, 5:6])
                yield

                # steps 6-7: tfd2
                pz2 = ppsc.tile([128, 512], F32, tag="psc")
                nc.tensor.matmul(pz2[:, 256:512], W("wda"), cst2, start=True, stop=False)
                nc.tensor.matmul(pz2[:, 256:512], W("wdb"), agd, start=False, stop=False)
                nc.tensor.matmul(pz2[:, 256:512], W("wds"), tfd0[:, 256:512], start=False, stop=False)
                yield
                nc.tensor.matmul(pz2[:, 256:512], W("wds"), tfd1[:, 256:512], start=False, stop=True)
                nc.tensor.matmul(pz2[:, 0:256], W("wf"), hu0, start=True, stop=False)
                nc.tensor.matmul(pz2[:, 0:256], W("wfs"), tfd0[:, 0:256], start=False, stop=False)
                nc.tensor.matmul(pz2[:, 0:256], W("wfs"), tfd1[:, 0:256], start=False, stop=True)
                tfd2 = pch.tile([128, 512], BF16, tag="tfd2")
                if zb:
                    nc.scalar.activation(tfd2, pz2, AFT.Tanh)
                else:
                    nc.scalar.activation(tfd2[:, 0:256], pz2[:, 0:256], AFT.Tanh, bias=fpack[:, 4:5])
                    nc.scalar.activation(tfd2[:, 256:512], pz2[:, 256:512], AFT.Tanh, bias=fpack[:, 5:6])
                tfds = [tfd0, tfd1, tfd2]
                yield

                stg = pstg.tile([128, 256], BF16, tag="stg")
                for w in range(4):
                    hi = w >= 2
                    R0, R1 = (64, 128) if hi else (0, 64)
                    b0 = 128 * (w % 2)
                    i64 = i64b if hi else i64a
                    ip1 = ip1b if hi else ip1a
                    sl = slice(b0, b0 + 128)
                    pe = ppe.tile([128, 256], F32, tag="pe")
                    nc.tensor.matmul(pe[:, 0:64], hl[R0:R1, sl], i64, start=True, stop=True)
                    nc.tensor.matmul(pe[:, 64:128], hu0[R0:R1, sl], i64, start=True, stop=False)
                    nc.tensor.matmul(pe[:, 64:128], tfds[0][R0:R1, sl], ip1, start=False, stop=False)
                    nc.tensor.matmul(pe[:, 64:128], tfds[1][R0:R1, sl], ip1, start=False, stop=False)
                    nc.tensor.matmul(pe[:, 64:128], tfds[2][R0:R1, sl], ip1, start=False, stop=True)
                    nc.tensor.matmul(pe[:, 128:192], cst2[R0:R1, sl], i64, start=True, stop=False)
                    nc.tensor.matmul(pe[:, 128:192], tfds[0][R0:R1, 256 + b0:256 + b0 + 128], ip1, start=False, stop=False)
                    nc.tensor.matmul(pe[:, 128:192], tfds[1][R0:R1, 256 + b0:256 + b0 + 128], ip1, start=False, stop=False)
                    nc.tensor.matmul(pe[:, 128:192], tfds[2][R0:R1, 256 + b0:256 + b0 + 128], ip1, start=False, stop=True)
                    t1 = pot.tile([128, 64], F32, tag="t1")
                    nc.scalar.activation(t1, pe[:, 0:64], AFT.Copy, scale=gk[:, 3 * w:3 * w + 1])
                    t2 = pot.tile([128, 64], F32, tag="t2")
                    nc.scalar.activation(t2, pe[:, 64:128], AFT.Copy, scale=gk[:, 3 * w + 1:3 * w + 2])
                    t3 = pot.tile([128, 64], F32, tag="t3")
                    nc.scalar.activation(t3, pe[:, 128:192], AFT.Copy, scale=gk[:, 3 * w + 2:3 * w + 3])
                    nc.gpsimd.tensor_tensor(t1, t1, t2, ALU.add)
                    nc.gpsimd.tensor_tensor(stg[:, 64 * w:64 * w + 64], t1, t3, ALU.add)
                    yield
                nc.gpsimd.dma_start(out=out_v[s], in_=stg.rearrange("p (w d) -> p w d", w=4))

            def advance(gen, n):
                if gen is None:
                    return None
                for _ in range(n):
                    try:
                        next(gen)
                    except StopIteration:
                        return None
                return gen

            pending = None
            for s in range(nsup):
                cstt = psup.tile([65, SB], BF16, tag="cstt")
                nc.scalar.dma_start(out=cstt, in_=cstt_d[:, s * SB:(s + 1) * SB])
                cst2 = psup.tile([128, 256], BF16, tag="cst2")
                nc.scalar.dma_start(out=cst2, in_=cst2_d[:, s * 256:(s + 1) * 256])
                mkf = psup.tile([104, 8 * MCH], BF16, tag="mkf")
                nc.gpsimd.dma_start(out=mkf, in_=mkf_d[s])
                agg3 = psup.tile([128, 768], BF16, tag="agg3")
                aggf2 = psup.tile([128, 256], BF16, tag="aggf2")

                # t8[b%128, 64*(b//128)+dout] = (cs@W1 + b_msg)[b, dout]
                pt = ppt.tile([128, 256], F32, tag="pt")
                for m in range(4):
                    nc.tensor.matmul(
                        pt[:, 64 * m:64 * m + 64],
                        cstt[:, 128 * m:128 * m + 128], w1b65,
                        start=True, stop=True,
                    )
                t8 = psup.tile([128, 256], BF16, tag="t8")
                nc.scalar.activation(t8, pt, AFT.Copy)

                for gg in range(SB // CB // GC):   # 2 groups per super
                    g = s * 2 + gg
                    mbl = pmb.tile([104, GC * NSF], F8E4, tag="mbl")
                    nc.sync.dma_start(out=mbl, in_=m8_d[g])
                    nsT = pns.tile([128, GC * 832], BF16, tag="nsT")
                    assert use_dmat, "PE-transpose fallback not implemented"
                    nc.sync.dma_start_transpose(out=nsT, in_=nst_d[g])
                    for cc in range(GC):
                        emit_chunk(s, gg, cc, mbl, nsT, t8, mkf, agg3, aggf2)
                        pending = advance(pending, 2 if gg * GC + cc >= 5 else 1)
                advance(pending, 99)
                pending = emit_chain(s, cst2, agg3, aggf2)
            advance(pending, 99)
    return nc


# ---------------- host-side packing ----------------

def host_pack(inputs, core, Bc):
    b0 = core * Bc
    ns = np.asarray(inputs["neighbor_states"][b0:b0 + Bc], np.float32)
    cs = np.asarray(inputs["current_state"][b0:b0 + Bc], np.float32)
    tier = np.asarray(inputs["tier"][b0:b0 + Bc], np.int32)

    nch = Bc // CB
    ngr = nch // GC
    nsup = Bc // SB

    w_mask = []
    for t in (0, 2, 1):
        m = (tier == t)
        w_mask.append(m.astype(np.float32) / np.maximum(m.sum(-1, keepdims=True), 1.0))
    w0, w2m, w1 = w_mask  # local, dist, func
    wmean = np.full_like(w0, 1.0 / K)

    nsr = ns.reshape(nch, 16, 4, K, D)
    ns_tok = np.ascontiguousarray(nsr.transpose(0, 2, 3, 1, 4)).reshape(nch, 104, NSF)

    # mk[c, (g k), q, m, g'] nonzero when g' == g
    wm3 = np.stack([w0, w2m, wmean], 0).reshape(3, nch, 16, 4, K)  # [m,c,q,g,k]
    M = np.zeros((nch, 4, K, 16, 3, 4), np.float32)
    for g in range(4):
        M[:, g, :, :, :, g] = wm3[:, :, :, g, :].transpose(1, 3, 2, 0)
    mk = M.reshape(nch, 104, MKF)

    w1r = w1.reshape(nch, 16, 4, K)
    F = np.zeros((nch, 4, K, 16, 4), np.float32)
    for g in range(4):
        F[:, g, :, :, g] = w1r[:, :, g, :].transpose(0, 2, 1)
    fse = F.reshape(nch, 104, FSF)

    m8 = np.ascontiguousarray(
        ns_tok.astype(ml_dtypes.float8_e4m3).reshape(ngr, GC, 104, NSF).transpose(0, 2, 1, 3)
    ).reshape(ngr, 104, GC * NSF)
    mkf = np.concatenate([mk, fse], axis=2).astype(ml_dtypes.bfloat16)   # [nch,104,256]
    mkf = np.ascontiguousarray(
        mkf.reshape(nsup, 8, 104, MCH).transpose(0, 2, 1, 3)
    ).reshape(nsup, 104, 8 * MCH)

    # pair-major for dma-transpose: [c, 832=(u,(g,k)), 128=(par,d)]
    T = nsr.reshape(nch, 8, 2, 4, K, D).transpose(0, 1, 3, 4, 2, 5)
    nst = np.ascontiguousarray(T).reshape(ngr, GC * 832, 128).astype(ml_dtypes.bfloat16)

    cst = cs.T.astype(np.float32)                      # [64, Bc]
    cstt = np.concatenate([cst, np.ones((1, Bc), np.float32)], 0).astype(ml_dtypes.bfloat16)
    cst2 = np.ascontiguousarray(
        cst.reshape(D, nsup, 2, 256).transpose(2, 0, 1, 3)
    ).reshape(128, nsup * 256).astype(ml_dtypes.bfloat16)

    def blk(w):
        z = np.zeros((128, 128), np.float32)
        z[0:64, 0:64] = w
        z[64:128, 64:128] = w
        return z

    wl = np.asarray(inputs["W_local"], np.float32)
    wu = np.asarray(inputs["W_upd"], np.float32)
    wf = np.asarray(inputs["W_fcnf"], np.float32)
    wd = np.asarray(inputs["W_dcnf"], np.float32)
    wg1 = np.asarray(inputs["W_g1"], np.float32)
    wg2 = np.asarray(inputs["W_g2"], np.float32)
    wmsg = np.asarray(inputs["W_msg"], np.float32)

    wpack = np.zeros((128, WPC), np.float32)
    for k, w in [("wl1", wl[:D]), ("wlb", wl[D:]), ("wu1", wu[:D]), ("wub", wu[D:]),
                 ("wf", wf), ("wfs", 0.1 * wf), ("wda", wd[:D]), ("wdb", wd[D:]),
                 ("wds", 0.1 * wd[:D])]:
        wpack[:, WB[k]:WB[k] + 128] = blk(w)
    g1a = np.zeros((128, 64), np.float32)
    g1a[0:64, 0:32] = wg1[:D]; g1a[64:128, 32:64] = wg1[:D]
    g1b = np.zeros((128, 64), np.float32)
    g1b[0:64, 0:32] = wg1[D:]; g1b[64:128, 32:64] = wg1[D:]
    wpack[:, WG1A:WG1A + 64] = g1a
    wpack[:, WG1B:WG1B + 64] = g1b
    wpack[0:32, WG2:WG2 + 3] = wg2
    wpack[32:64, WG2 + 3:WG2 + 6] = wg2
    fpack = np.zeros((128, 8), np.float32)
    bg1 = np.asarray(inputs["b_g1"], np.float32)
    fpack[0:32, 0] = bg1; fpack[32:64, 0] = bg1
    fpack[64:67, 1] = np.asarray(inputs["b_g2"], np.float32)
    bl = np.asarray(inputs["b_local"], np.float32)
    bu = np.asarray(inputs["b_upd"], np.float32)
    bfc = np.asarray(inputs["b_fcnf"], np.float32)
    bdc = np.asarray(inputs["b_dcnf"], np.float32)
    fpack[0:64, 2] = bl; fpack[64:128, 2] = bl
    fpack[0:64, 3] = bu; fpack[64:128, 3] = bu
    fpack[0:64, 4] = bfc; fpack[64:128, 4] = bfc
    fpack[0:64, 5] = bdc; fpack[64:128, 5] = bdc

    bpack = np.zeros((128, BPC), np.float32)
    bpack[:, BW2:BW2 + 128] = blk(wmsg[D:])
    bpack[0:64, BW1B:BW1B + 64] = wmsg[:D]
    bpack[64, BW1B:BW1B + 64] = np.asarray(inputs["b_msg"], np.float32)
    ii = np.eye(64, dtype=np.float32)
    bpack[0:64, BI64:BI64 + 64] = ii; bpack[64:128, BI64:BI64 + 64] = ii
    bpack[0:64, BIP1:BIP1 + 64] = 0.1 * ii; bpack[64:128, BIP1:BIP1 + 64] = 0.1 * ii
    e3 = np.eye(3, dtype=np.float32)
    bpack[64:67, BEY3:BEY3 + 3] = e3
    tp = np.arange(104) // K            # t' // 26
    for ph in range(2):
        for u in range(8):
            for par in range(2):
                si = SELB + 104 * (((ph * 8 + u) * 2) + par)
                rows = 64 * ph + 8 * u + 4 * par + tp
                bpack[rows, si + np.arange(104)] = 1.0

    m = {
        "m8": m8,
        "mkf": mkf,
        "nst": nst,
        "cstt": cstt,
        "cst2": cst2,
        "wpack": wpack.astype(ml_dtypes.bfloat16),
        "fpack": fpack,
        "bpack": bpack.astype(ml_dtypes.bfloat16),
    }
    return {k: np.ascontiguousarray(v) for k, v in m.items()}


def _zb(inputs):
    return all(
        not np.any(np.asarray(inputs[k]))
        for k in ("b_local", "b_upd", "b_fcnf", "b_dcnf", "b_g2")
    )


_CACHE = {}


def _get_program(Bc, zb=True, use_dmat=True):
    key = (Bc, zb, use_dmat)
    if key not in _CACHE:
        nc = build_program(Bc, zb=zb, use_dmat=use_dmat)
        _split_waits(nc)
        _CACHE[key] = nc
    return _CACHE[key]


USE_DMAT = os.environ.get("K2_NO_DMAT", "") == ""


def run(inputs, trace=False):
    B = inputs["current_state"].shape[0]
    Bc = B // NCORES
    nc = _get_program(Bc, zb=_zb(inputs), use_dmat=USE_DMAT)
    in_maps = [host_pack(inputs, core, Bc) for core in range(NCORES)]
    res = run_bass_kernel_spmd(
        nc, in_maps, core_ids=list(range(NCORES)), trace=trace
    )
    out = np.concatenate([np.asarray(r["out"]).astype(np.float32) for r in res.results], axis=0)
    return out, res


def kernel(**inputs):
    out, _ = run(inputs)
    return out
